# revision 1
# baseline (speedup 1.0000x reference)
"""GCN layer (2x segment-sum aggregate + linear) on 8 Trainium2 NeuronCores.

Sharding: nodes (and their incident edges, by dst) are partitioned across the
8 cores; the feature table is replicated in each core's HBM.

Per-core algorithm, per aggregation round:
  - edges are grouped on the host into 128-edge "tiles"; each tile's dsts all
    fall in one 128-node block of the (permuted) local node space
  - dma_gather pulls the 128 src rows of each tile from the HBM table into
    SBUF (int16 indices => edges are pre-split into src<32768 / src>=32768
    streams; the high stream gathers from a +32768 table view; max 1024
    indices per instruction on HW)
  - a one-hot selection matrix S (S[edge, r] = dst_local==r) is built on DVE
    as one grouped is_equal per (block, stream) against an iota matrix
  - PE accumulates psum[feat, r] += E_tile^T @ S_tile over the block's tiles
  - results are staged in SBUF and flushed with one DMA per round;
    round 1 AllGathers the full h1 table; round 2 feeds psum straight into
    the 128x128 linear (+bias)

The local node ids are permuted on the host (balanced bin packing) so that
every 128-node block needs exactly 9 low + 5 high tiles on every core: the
compiled program is identical across cores (SPMD), only tensors differ.
kernel() un-permutes when assembling the full output.
"""

import numpy as np

import concourse.bass as bass
import concourse.bacc as bacc
import concourse.mybir as mybir
import concourse.tile as tile
from concourse.bass_utils import run_bass_kernel_spmd
from concourse.masks import make_identity

# ---- problem constants (hardcoded per contest contract) ----
N_NODES = 50000
D = 128
NCORES = 8
PER = N_NODES // NCORES          # 6250 real nodes per core
HSPLIT = 32768                   # int16 index limit split point
NBIN = 49                        # 128-node blocks per core
NSLAB = NBIN * 128               # 6272 padded local node slots
NFULL = NCORES * NSLAB           # 50176 rows in the gathered h1 table
T_LO = 9                         # low-stream tiles per block
T_HI = 5                         # high-stream tiles per block
NT_LO = NBIN * T_LO              # 441
NT_HI = NBIN * T_HI              # 245
CH = 512                         # edges per gather chunk (1024 hits a slow packet path; >1024 crashes)
TPC = CH // 128                  # tiles per chunk
NCH_LO = (NT_LO + TPC - 1) // TPC
NCH_HI = (NT_HI + TPC - 1) // TPC
NT = NT_LO + NT_HI               # tiles per round


def set_ch(ch: int):
    """Set the gather chunk size and recompute derived constants."""
    global CH, TPC, NCH_LO, NCH_HI
    CH = ch
    TPC = CH // 128
    NCH_LO = (NT_LO + TPC - 1) // TPC
    NCH_HI = (NT_HI + TPC - 1) // TPC
PROBE_MODE = None                # None | "gather_only" | "seq_dma" | "no_gather"
NQ = 1                           # SWDGE queues to spread gathers over (1-4)
GATHER_BF16 = False              # gather/store tables in bf16 (f32 otherwise)
SINGLE_PACKET = True             # dma_gather single_packet flag


def _mid_bcast(ap, k):
    """[128, r] AP -> [128, k, r] with the middle dim broadcast."""
    return bass.AP(ap.tensor, ap.offset, [ap.ap[0], [0, k], ap.ap[1]])


def _build_nc(repeat: int = 1, timing_loop: int | None = None,
              mode: str = "full"):
    nc = bacc.Bacc(
        "TRN2",
        target_bir_lowering=False,
        debug=False,
        num_devices=NCORES,
        num_swdge_queues=NQ,
    )
    f32, i16 = mybir.dt.float32, mybir.dt.int16
    gdt = mybir.dt.bfloat16 if GATHER_BF16 else f32

    feature = nc.dram_tensor("feature", [N_NODES, D], gdt, kind="ExternalInput")
    g1l = nc.dram_tensor("g1l", [128, NCH_LO * CH // 16], i16, kind="ExternalInput")
    g1h = nc.dram_tensor("g1h", [128, NCH_HI * CH // 16], i16, kind="ExternalInput")
    g2l = nc.dram_tensor("g2l", [128, NCH_LO * CH // 16], i16, kind="ExternalInput")
    g2h = nc.dram_tensor("g2h", [128, NCH_HI * CH // 16], i16, kind="ExternalInput")
    # negated local dst (within-block) per tile slot, rounds 1&2: [128, NT]
    nd1 = nc.dram_tensor("nd1", [128, NT], f32, kind="ExternalInput")
    nd2 = nc.dram_tensor("nd2", [128, NT], f32, kind="ExternalInput")
    w_in = nc.dram_tensor("w_in", [D, D], f32, kind="ExternalInput")
    b_in = nc.dram_tensor("b_in", [D, 1], f32, kind="ExternalInput")
    out_t = nc.dram_tensor("out_t", [D, NSLAB], f32, kind="ExternalOutput")

    with tile.TileContext(nc) as tc:
        with (
            tc.tile_pool(name="const", bufs=1) as cpool,
            tc.tile_pool(name="idx", bufs=1) as ipool,
            tc.tile_pool(name="ebuf", bufs=6) as epool,
            tc.tile_pool(name="sel", bufs=4) as spool,
            tc.tile_pool(name="fl", bufs=4) as fpool,
            tc.tile_pool(name="stg", bufs=1) as stpool,
            tc.tile_pool(name="ps", bufs=3, space="PSUM") as pspool,
            tc.tile_pool(name="ps2", bufs=2, space="PSUM") as ps2pool,
            tc.tile_pool(name="dram", bufs=1, space="DRAM") as dpool,
        ):
            gl_t = {r: ipool.tile([128, NCH_LO * CH // 16], i16,
                                  tag=f"g{r}l", name=f"g{r}l_t") for r in (1, 2)}
            gh_t = {r: ipool.tile([128, NCH_HI * CH // 16], i16,
                                  tag=f"g{r}h", name=f"g{r}h_t") for r in (1, 2)}
            nd_t = {r: ipool.tile([128, NT], f32, tag=f"nd{r}",
                                  name=f"nd{r}_t") for r in (1, 2)}
            nc.sync.dma_start(out=gl_t[1][:], in_=g1l[:])
            nc.sync.dma_start(out=gh_t[1][:], in_=g1h[:])
            nc.sync.dma_start(out=gl_t[2][:], in_=g2l[:])
            nc.sync.dma_start(out=gh_t[2][:], in_=g2h[:])
            nc.sync.dma_start(out=nd_t[1][:], in_=nd1[:])
            nc.sync.dma_start(out=nd_t[2][:], in_=nd2[:])

            w_t = cpool.tile([D, D], f32)
            b_t = cpool.tile([D, 1], f32)
            nc.sync.dma_start(out=w_t[:], in_=w_in[:])
            nc.sync.dma_start(out=b_t[:], in_=b_in[:])
            # neg_iota[p, r] = -r  (f32 ints <=128: exact)
            neg_iota = cpool.tile([128, 128], f32)
            nc.gpsimd.iota(neg_iota[:], pattern=[[-1, 128]], base=0,
                           channel_multiplier=0,
                           allow_small_or_imprecise_dtypes=True)

            h1part = dpool.tile([NSLAB, D], gdt)

            def one_round(rnd, table, table_hi):
                ebufs = {}
                qctr = [0]

                def ensure_chunk(stream, c):
                    key = (stream, c)
                    if key in ebufs:
                        return ebufs[key]
                    eb = epool.tile([128, TPC * D], gdt, tag="ebuf")
                    g = gl_t[rnd] if stream == 0 else gh_t[rnd]
                    tab = table if stream == 0 else table_hi
                    if PROBE_MODE == "seq_dma":
                        nc.sync.dma_start(out=eb[:], in_=tab[0:TPC * 128, :])
                    elif PROBE_MODE == "no_gather":
                        pass
                    else:
                        nc.gpsimd.dma_gather(
                            eb[:].rearrange("p (n d) -> p n d", d=D),
                            tab,
                            g[:, c * (CH // 16):(c + 1) * (CH // 16)],
                            num_idxs=CH, num_idxs_reg=CH,
                            elem_size=D, elem_step=D,
                            queue_num=qctr[0] % NQ,
                            single_packet=SINGLE_PACKET)
                        qctr[0] += 1
                    ebufs[key] = eb
                    return eb

                stage = stpool.tile([128, NBIN * 128], gdt if rnd == 1 else f32,
                                    tag=f"stage{rnd}")
                for b in range(NBIN):
                    # grouped one-hot builds: S[edge, t, r] = (dstL[edge,t]==r)
                    S_lo = spool.tile([128, T_LO * 128], gdt, tag="Slo")
                    nc.vector.tensor_tensor(
                        out=S_lo[:].rearrange("p (t r) -> p t r", r=128),
                        in0=nd_t[rnd][:, b * T_LO:(b + 1) * T_LO]
                            .to_broadcast([128, T_LO, 128]),
                        in1=_mid_bcast(neg_iota[:], T_LO),
                        op=mybir.AluOpType.is_equal)
                    S_hi = spool.tile([128, T_HI * 128], gdt, tag="Shi")
                    c0 = NT_LO + b * T_HI
                    nc.vector.tensor_tensor(
                        out=S_hi[:].rearrange("p (t r) -> p t r", r=128),
                        in0=nd_t[rnd][:, c0:c0 + T_HI]
                            .to_broadcast([128, T_HI, 128]),
                        in1=_mid_bcast(neg_iota[:], T_HI),
                        op=mybir.AluOpType.is_equal)

                    ps = pspool.tile([128, 128], f32, tag="agg")
                    tiles = [(0, b * T_LO + j, S_lo, j) for j in range(T_LO)] + \
                            [(1, b * T_HI + j, S_hi, j) for j in range(T_HI)]
                    if PROBE_MODE == "gather_only":
                        for j, (stream, t, S, jj) in enumerate(tiles):
                            ensure_chunk(stream, t // TPC)
                        nc.tensor.matmul(ps[:], lhsT=S_lo[:, 0:128],
                                         rhs=S_hi[:, 0:128], start=True, stop=True)
                    else:
                        for j, (stream, t, S, jj) in enumerate(tiles):
                            eb = ensure_chunk(stream, t // TPC)
                            st = t % TPC
                            ebs = eb[:, st * D:(st + 1) * D]        # [edge, feat]
                            Ssl = S[:, jj * 128:(jj + 1) * 128]     # [edge, r]
                            nc.tensor.matmul(
                                ps[:],
                                # rnd1: out[r, feat] (stage layout direct);
                                # rnd2: out[feat, r] (feeds W matmul)
                                lhsT=Ssl if rnd == 1 else ebs,
                                rhs=ebs if rnd == 1 else Ssl,
                                start=(j == 0), stop=(j == len(tiles) - 1))
                    if rnd == 1:
                        nc.vector.tensor_copy(
                            stage[:, b * 128:(b + 1) * 128], ps[:])
                    else:
                        h_sb = fpool.tile([128, 128], f32, tag="hsb")
                        nc.scalar.copy(h_sb[:], ps[:])
                        o_ps = ps2pool.tile([128, 128], f32, tag="ops")
                        nc.tensor.matmul(o_ps[:], lhsT=w_t[:], rhs=h_sb[:],
                                         start=True, stop=True)
                        nc.vector.tensor_scalar_add(
                            stage[:, b * 128:(b + 1) * 128], o_ps[:], b_t[:, 0:1])
                # single flush DMA per round
                if rnd == 1:
                    nc.sync.dma_start(
                        out=h1part[:].rearrange("(b p) d -> p b d", p=128),
                        in_=stage[:].rearrange("p (b d) -> p b d", d=128))
                else:
                    nc.sync.dma_start(out=out_t[:], in_=stage[:])

            def allgather(rep, do_ag=True):
                # Shared output allows peers to RDMA-write directly (fast
                # path), but a Shared tensor takes exactly one writer inst:
                # allocate one per repeat iteration.
                h1full = dpool.tile([NFULL, D], gdt, addr_space="Shared",
                                    tag=f"h1full{rep}", name=f"h1full{rep}")
                if do_ag:
                    nc.gpsimd.collective_compute(
                        "AllGather", mybir.AluOpType.bypass,
                        replica_groups=[list(range(NCORES))],
                        ins=[h1part.opt()], outs=[h1full.opt()])
                return h1full

            if timing_loop is None:
                for _rep in range(repeat):
                    if mode == "agonly":
                        allgather(_rep)
                        continue
                    one_round(1, feature[:], feature[HSPLIT:, :])
                    h1full = allgather(_rep, do_ag=(mode == "full"))
                    one_round(2, h1full[:], h1full[HSPLIT:, :])
            else:
                # timing build: AllGather once (fills h1full), then loop rounds
                one_round(1, feature[:], feature[HSPLIT:, :])
                h1full = allgather(0)
                with tc.For_i(0, timing_loop, 1):
                    one_round(1, feature[:], feature[HSPLIT:, :])
                    one_round(2, h1full[:], h1full[HSPLIT:, :])
    nc.compile()
    return nc


def _build_nc_v2(repeat: int = 1, mode: str = "full"):
    """SBUF-resident bf16 table + transpose-mode SBUF-source gathers.

    Table layout (both rounds): row r lives at SBUF partition r%128, free
    byte offset (r//128)*256 — so a gather index is just the row id, and the
    round-1 stage buffer is already in this layout (flat flush, flat reload).
    Per 128-edge tile:
      Zmm:  z[e, f] = matmul(lhsT=E'[f, e], rhs=R)   (R = I round 1, W round 2
                                                       -> transpose [+ linear])
      copy: z_sb <- z (ACT, PSUM->SBUF)
      agg:  rnd1 ps[r, f] += matmul(lhsT=S[e, r], rhs=z_sb[e, f])
            rnd2 ps[of, r] += matmul(lhsT=z_sb[e, of], rhs=S[e, r])
    """
    nc = bacc.Bacc(
        "TRN2",
        target_bir_lowering=False,
        debug=False,
        num_devices=NCORES,
        num_swdge_queues=NQ,
    )
    f32, i16, bf16 = mybir.dt.float32, mybir.dt.int16, mybir.dt.bfloat16
    NRANK = NFULL // 128             # 392 ranks of 128 rows

    feature = nc.dram_tensor("feature", [128, NFULL], bf16, kind="ExternalInput")
    g1l = nc.dram_tensor("g1l", [128, NCH_LO * CH // 16], i16, kind="ExternalInput")
    g1h = nc.dram_tensor("g1h", [128, NCH_HI * CH // 16], i16, kind="ExternalInput")
    g2l = nc.dram_tensor("g2l", [128, NCH_LO * CH // 16], i16, kind="ExternalInput")
    g2h = nc.dram_tensor("g2h", [128, NCH_HI * CH // 16], i16, kind="ExternalInput")
    nd1 = nc.dram_tensor("nd1", [128, NT], f32, kind="ExternalInput")
    nd2 = nc.dram_tensor("nd2", [128, NT], f32, kind="ExternalInput")
    w_in = nc.dram_tensor("w_in", [D, D], f32, kind="ExternalInput")
    b_in = nc.dram_tensor("b_in", [D, 1], f32, kind="ExternalInput")
    out_t = nc.dram_tensor("out_t", [D, NSLAB], f32, kind="ExternalOutput")

    with tile.TileContext(nc) as tc:
        with (
            tc.tile_pool(name="const", bufs=1) as cpool,
            tc.tile_pool(name="idx", bufs=1) as ipool,
            tc.tile_pool(name="tblp", bufs=1) as tpool,
            tc.tile_pool(name="ebuf", bufs=6) as epool,
            tc.tile_pool(name="zb", bufs=6) as zpool,
            tc.tile_pool(name="sel", bufs=4) as spool,
            tc.tile_pool(name="stg", bufs=1) as stpool,
            tc.tile_pool(name="psz", bufs=4, space="PSUM") as pszpool,
            tc.tile_pool(name="ps", bufs=3, space="PSUM") as pspool,
            tc.tile_pool(name="dram", bufs=1, space="DRAM") as dpool,
        ):
            gl_t = {r: ipool.tile([128, NCH_LO * CH // 16], i16,
                                  tag=f"g{r}l", name=f"g{r}l_t") for r in (1, 2)}
            gh_t = {r: ipool.tile([128, NCH_HI * CH // 16], i16,
                                  tag=f"g{r}h", name=f"g{r}h_t") for r in (1, 2)}
            nd_t = {r: ipool.tile([128, NT], f32, tag=f"nd{r}",
                                  name=f"nd{r}_t") for r in (1, 2)}
            nc.sync.dma_start(out=gl_t[1][:], in_=g1l[:])
            nc.sync.dma_start(out=gh_t[1][:], in_=g1h[:])
            nc.sync.dma_start(out=gl_t[2][:], in_=g2l[:])
            nc.sync.dma_start(out=gh_t[2][:], in_=g2h[:])
            nc.sync.dma_start(out=nd_t[1][:], in_=nd1[:])
            nc.sync.dma_start(out=nd_t[2][:], in_=nd2[:])

            w_t = cpool.tile([D, D], f32)
            nc.sync.dma_start(out=w_t[:], in_=w_in[:])
            w_bf = cpool.tile([D, D], bf16)
            nc.vector.tensor_copy(w_bf[:], w_t[:])
            b_t = cpool.tile([D, 1], f32)
            nc.sync.dma_start(out=b_t[:], in_=b_in[:])
            ident = cpool.tile([128, 128], bf16)
            make_identity(nc, ident[:])
            neg_iota = cpool.tile([128, 128], f32)
            nc.gpsimd.iota(neg_iota[:], pattern=[[-1, 128]], base=0,
                           channel_multiplier=0,
                           allow_small_or_imprecise_dtypes=True)

            h1part = dpool.tile([128, NSLAB], bf16)
            h1full = dpool.tile([NCORES * 128, NSLAB], bf16)
            tbl = tpool.tile([128, NFULL], bf16)

            def load_table(rnd):
                if rnd == 1:
                    nc.sync.dma_start(out=tbl[:], in_=feature[:])
                else:
                    nc.sync.dma_start(
                        out=tbl[:].rearrange("p (c f) -> p c f", c=NCORES),
                        in_=h1full[:].rearrange("(c p) f -> p c f", p=128))

            def one_round(rnd):
                ebufs = {}
                qctr = [0]

                def ensure_chunk(stream, c):
                    key = (stream, c)
                    if key in ebufs:
                        return ebufs[key]
                    eb = epool.tile([128, CH], bf16, tag="ebuf")
                    if PROBE_MODE == "no_gather":
                        nc.vector.memset(eb[:], 0.0)
                    else:
                        g = gl_t[rnd] if stream == 0 else gh_t[rnd]
                        src = tbl[:] if stream == 0 else tbl[:, HSPLIT:]
                        nc.gpsimd.dma_gather(
                            eb[:].rearrange("p (o e) -> p o e", o=1),
                            src,
                            g[:, c * (CH // 16):(c + 1) * (CH // 16)],
                            num_idxs=CH, num_idxs_reg=CH,
                            elem_size=D, transpose=True,
                            sbuf_tokens_per_rank=128,
                            sbuf_free_dim_per_rank=256,
                            queue_num=qctr[0] % NQ,
                            single_packet=SINGLE_PACKET)
                        qctr[0] += 1
                    ebufs[key] = eb
                    return eb

                stage = stpool.tile([128, NSLAB], bf16 if rnd == 1 else f32,
                                    tag=f"stage{rnd}")
                for b in range(NBIN):
                    S_lo = spool.tile([128, T_LO * 128], bf16, tag="Slo")
                    nc.vector.tensor_tensor(
                        out=S_lo[:].rearrange("p (t r) -> p t r", r=128),
                        in0=nd_t[rnd][:, b * T_LO:(b + 1) * T_LO]
                            .to_broadcast([128, T_LO, 128]),
                        in1=_mid_bcast(neg_iota[:], T_LO),
                        op=mybir.AluOpType.is_equal)
                    S_hi = spool.tile([128, T_HI * 128], bf16, tag="Shi")
                    c0 = NT_LO + b * T_HI
                    nc.vector.tensor_tensor(
                        out=S_hi[:].rearrange("p (t r) -> p t r", r=128),
                        in0=nd_t[rnd][:, c0:c0 + T_HI]
                            .to_broadcast([128, T_HI, 128]),
                        in1=_mid_bcast(neg_iota[:], T_HI),
                        op=mybir.AluOpType.is_equal)

                    ps = pspool.tile([128, 128], f32, tag="agg")
                    tiles = [(0, b * T_LO + j, S_lo, j) for j in range(T_LO)] + \
                            [(1, b * T_HI + j, S_hi, j) for j in range(T_HI)]
                    if PROBE_MODE == "gather_only":
                        for j, (stream, t, S, jj) in enumerate(tiles):
                            ensure_chunk(stream, t // TPC)
                        nc.tensor.matmul(ps[:], lhsT=S_lo[:, 0:128],
                                         rhs=S_hi[:, 0:128], start=True,
                                         stop=True)
                        tiles = []
                    for j, (stream, t, S, jj) in enumerate(tiles):
                        eb = ensure_chunk(stream, t // TPC)
                        st = t % TPC
                        zp = pszpool.tile([128, 128], f32, tag="zps")
                        nc.tensor.matmul(
                            zp[:],
                            lhsT=eb[:, st * 128:(st + 1) * 128],
                            rhs=ident[:] if rnd == 1 else w_bf[:],
                            start=True, stop=True)
                        z_sb = zpool.tile([128, 128], bf16, tag="zsb")
                        nc.scalar.copy(z_sb[:], zp[:])
                        Ssl = S[:, jj * 128:(jj + 1) * 128]
                        nc.tensor.matmul(
                            ps[:],
                            lhsT=Ssl if rnd == 1 else z_sb[:],
                            rhs=z_sb[:] if rnd == 1 else Ssl,
                            start=(j == 0), stop=(j == len(tiles) - 1))
                    if rnd == 1:
                        nc.vector.tensor_copy(
                            stage[:, b * 128:(b + 1) * 128], ps[:])
                    else:
                        nc.vector.tensor_scalar_add(
                            stage[:, b * 128:(b + 1) * 128], ps[:], b_t[:, 0:1])
                if rnd == 1:
                    nc.sync.dma_start(out=h1part[:], in_=stage[:])
                else:
                    nc.sync.dma_start(out=out_t[:], in_=stage[:])

            def allgather():
                nc.gpsimd.collective_compute(
                    "AllGather", mybir.AluOpType.bypass,
                    replica_groups=[list(range(NCORES))],
                    ins=[h1part.opt()], outs=[h1full.opt()])

            for _rep in range(repeat):
                if mode == "agonly":
                    allgather()
                    continue
                load_table(1)
                one_round(1)
                if mode == "full":
                    allgather()
                    load_table(2)
                one_round(2)
    nc.compile()
    return nc


VARIANT = "v1"
_NC_CACHE: dict = {}


def get_nc(repeat: int = 1):
    key = (VARIANT, repeat, NQ, GATHER_BF16, SINGLE_PACKET)
    if key not in _NC_CACHE:
        _NC_CACHE[key] = (_build_nc_v2(repeat) if VARIANT == "v2"
                          else _build_nc(repeat))
    return _NC_CACHE[key]


def _wrap_idx(idx: np.ndarray) -> np.ndarray:
    """[n] -> [128, n//16] int16 wrapped layout (16-partition wrap, 8x
    replicated for the Q7 cores)."""
    n = idx.shape[0]
    w = idx.reshape(n // 16, 16).T.astype(np.int16)
    return np.ascontiguousarray(np.tile(w, (8, 1)))


def _pack_bins(deg_lo: np.ndarray, deg: np.ndarray):
    """Balanced snake packing of PER nodes into NBIN bins (<=128 nodes each).
    Returns perm: perm[orig_local] = bin*128 + slot."""
    order = np.argsort(-(deg_lo * 2 + deg), kind="stable")
    bins = [[] for _ in range(NBIN)]
    pos, fwd = 0, True
    for n in order:
        for _ in range(NBIN + 1):
            if len(bins[pos]) < 128:
                break
            pos, fwd = _step(pos, fwd, NBIN)
        bins[pos].append(n)
        pos, fwd = _step(pos, fwd, NBIN)
    perm = np.empty(PER, np.int64)
    for bi, members in enumerate(bins):
        for sl, n in enumerate(members):
            perm[n] = bi * 128 + sl
    return perm


def _step(pos, fwd, nbins):
    if fwd:
        if pos + 1 >= nbins:
            return pos, False
        return pos + 1, True
    if pos - 1 < 0:
        return pos, True
    return pos - 1, False


def _build_round_tensors(g_src: np.ndarray, dslot: np.ndarray):
    """Per-edge gather ids + permuted local dst slots -> (gl, gh, nd)."""
    blk = dslot >> 7
    dstL = dslot & 127
    is_lo = g_src < HSPLIT

    g_lo = np.zeros(NCH_LO * CH, np.int64)
    g_hi = np.zeros(NCH_HI * CH, np.int64)
    ndl = np.full((128, NT), 1.0, np.float32)   # -dstL; pad = +1 (never matches)
    for b in range(NBIN):
        in_b = blk == b
        for stream, (g_arr, t0, t_n, base_off) in enumerate(
                [(g_lo, b * T_LO, T_LO, 0), (g_hi, b * T_HI, T_HI, NT_LO)]):
            m = in_b & (is_lo if stream == 0 else ~is_lo)
            gs = g_src[m] - (0 if stream == 0 else HSPLIT)
            dl = dstL[m]
            cnt = gs.shape[0]
            assert cnt <= t_n * 128, (b, stream, cnt, t_n * 128)
            sl = t0 * 128
            g_arr[sl: sl + cnt] = gs
            col = np.arange(cnt) // 128 + t0 + base_off
            row = np.arange(cnt) % 128
            ndl[row, col] = -dl.astype(np.float32)
    return (_wrap_idx(g_lo), _wrap_idx(g_hi), ndl)


def prep_core_inputs(feature, W, b, src, dst):
    feature = np.ascontiguousarray(np.asarray(feature, dtype=np.float32))
    W = np.ascontiguousarray(np.asarray(W, dtype=np.float32))
    b = np.asarray(b, dtype=np.float32).reshape(D, 1)
    src = np.asarray(src).astype(np.int64)
    dst = np.asarray(dst).astype(np.int64)

    owner = dst // PER
    deg_lo_all = np.bincount(dst[src < HSPLIT], minlength=N_NODES)
    deg_all = np.bincount(dst, minlength=N_NODES)
    perms = []
    for c in range(NCORES):
        dlo = deg_lo_all[c * PER:(c + 1) * PER]
        dg = deg_all[c * PER:(c + 1) * PER]
        perms.append(_pack_bins(dlo.astype(np.int64), dg.astype(np.int64)))

    permg = np.empty(N_NODES, np.int64)
    for c in range(NCORES):
        permg[c * PER:(c + 1) * PER] = c * NSLAB + perms[c]

    if VARIANT == "v2":
        # wrapped layout: feat_w[p, (r//128)*128 + f] = feature[r, f], r%128==p
        fpad = np.zeros((NFULL, D), np.float32)
        fpad[:N_NODES] = feature
        feat_in = np.ascontiguousarray(
            fpad.reshape(NFULL // 128, 128, D).transpose(1, 0, 2)
                .reshape(128, NFULL)).astype(mybir.dt.np(mybir.dt.bfloat16))
    else:
        feat_in = (feature.astype(mybir.dt.np(mybir.dt.bfloat16))
                   if GATHER_BF16 else feature)
    in_maps = []
    for c in range(NCORES):
        sel = owner == c
        es, ed = src[sel], dst[sel] - c * PER
        dslot = perms[c][ed]
        g1l_, g1h_, nd1_ = _build_round_tensors(es, dslot)
        g2l_, g2h_, nd2_ = _build_round_tensors(permg[es], dslot)
        in_maps.append({
            "feature": feat_in,
            "g1l": g1l_, "g1h": g1h_, "nd1": nd1_,
            "g2l": g2l_, "g2h": g2h_, "nd2": nd2_,
            "w_in": W, "b_in": b,
        })
    return in_maps, perms


def assemble(results, perms) -> np.ndarray:
    out = np.empty((N_NODES, D), np.float32)
    for c in range(NCORES):
        ot = np.asarray(results[c]["out_t"])       # [D, NSLAB]
        out[c * PER:(c + 1) * PER, :] = ot.T[perms[c], :]
    return out


def kernel(feature, W, b, src, dst) -> np.ndarray:
    nc = get_nc(repeat=1)
    in_maps, perms = prep_core_inputs(feature, W, b, src, dst)
    res = run_bass_kernel_spmd(nc, in_maps, core_ids=list(range(NCORES)))
    return assemble(res.results, perms)



# revision 9
# speedup vs baseline: 91.4090x; 91.4090x over previous
"""GCN layer (2x segment-sum aggregate + linear) on 8 Trainium2 NeuronCores.

Sharding: nodes (and their incident edges, by dst) are partitioned across the
8 cores; the feature table is replicated in each core's HBM.

Perf-critical configuration (measured via 1000-iteration HW-loop differential
timing; each knob validated on hardware):
  - bf16 tables + bf16 gathers (f32 psum accumulation keeps rel err ~2.5e-3):
    ~4.8x over f32 end-to-end, and makes the AllGather ~free vs f32.
  - single_packet=False: multi-packet SWDGE descriptor streams let the DMA
    engines interleave a gather's 256B random reads, ~4x.
  - 4 SWDGE queues, gathers round-robined: ~2x (costs ~0.5ms on the
    collective, but the gather win dominates).
  - 24 chunk buffers of prefetch depth (~14 dst-block bins of lookahead):
    ~4x over 6 buffers; gathers then fully overlap compute.
  - CH=1024 indices per dma_gather: halves the ~1us fixed SWDGE cost/inst.

Per-core algorithm, per aggregation round:
  - edges are grouped on the host into 128-edge "tiles"; each tile's dsts all
    fall in one 128-node block of the (permuted) local node space
  - dma_gather pulls the 128 src rows of each tile from the HBM table into
    SBUF (int16 indices => edges are pre-split into src<32768 / src>=32768
    streams; the high stream gathers from a +32768 table view; max 1024
    indices per instruction on HW)
  - a one-hot selection matrix S (S[edge, r] = dst_local==r) is built on DVE
    as one grouped is_equal per (block, stream) against an iota matrix
  - PE accumulates psum[feat, r] += E_tile^T @ S_tile over the block's tiles
  - results are staged in SBUF and flushed with one DMA per round;
    round 1 AllGathers the full h1 table; round 2 feeds psum straight into
    the 128x128 linear (+bias)

The local node ids are permuted on the host (balanced bin packing) so that
every 128-node block needs exactly 9 low + 5 high tiles on every core: the
compiled program is identical across cores (SPMD), only tensors differ.
kernel() un-permutes when assembling the full output.
"""

import numpy as np

import concourse.bass as bass
import concourse.bacc as bacc
import concourse.mybir as mybir
import concourse.tile as tile
from concourse.bass_utils import run_bass_kernel_spmd
from concourse.masks import make_identity

# ---- problem constants (hardcoded per contest contract) ----
N_NODES = 50000
D = 128
NCORES = 8
PER = N_NODES // NCORES          # 6250 real nodes per core
HSPLIT = 32768                   # int16 index limit split point
NBIN = 49                        # 128-node blocks per core
NSLAB = NBIN * 128               # 6272 padded local node slots
NFULL = NCORES * NSLAB           # 50176 rows in the gathered h1 table
T_LO = 9                         # low-stream tiles per block
T_HI = 5                         # high-stream tiles per block
NT_LO = NBIN * T_LO              # 441
NT_HI = NBIN * T_HI              # 245
CH = 1024                        # edges per gather chunk (max 1024 idxs/instruction)
TPC = CH // 128                  # tiles per chunk
NCH_LO = (NT_LO + TPC - 1) // TPC
NCH_HI = (NT_HI + TPC - 1) // TPC
NT = NT_LO + NT_HI               # tiles per round


def set_ch(ch: int):
    """Set the gather chunk size and recompute derived constants."""
    global CH, TPC, NCH_LO, NCH_HI
    CH = ch
    TPC = CH // 128
    NCH_LO = (NT_LO + TPC - 1) // TPC
    NCH_HI = (NT_HI + TPC - 1) // TPC
PROBE_MODE = None                # None | "gather_only" | "seq_dma" | "no_gather"
NQ = 4                           # SWDGE queues to spread gathers over (1-4)
EBUFS = 24                       # gather chunk buffers (prefetch depth)
GATHER_BF16 = True               # gather/store tables in bf16 (f32 otherwise)
SINGLE_PACKET = False            # dma_gather single_packet flag


def _mid_bcast(ap, k):
    """[128, r] AP -> [128, k, r] with the middle dim broadcast."""
    return bass.AP(ap.tensor, ap.offset, [ap.ap[0], [0, k], ap.ap[1]])


def _build_nc(repeat: int = 1, timing_loop: int | None = None,
              mode: str = "full"):
    nc = bacc.Bacc(
        "TRN2",
        target_bir_lowering=False,
        debug=False,
        num_devices=NCORES,
        num_swdge_queues=NQ,
    )
    f32, i16 = mybir.dt.float32, mybir.dt.int16
    gdt = mybir.dt.bfloat16 if GATHER_BF16 else f32

    feature = nc.dram_tensor("feature", [N_NODES, D], gdt, kind="ExternalInput")
    g1l = nc.dram_tensor("g1l", [128, NCH_LO * CH // 16], i16, kind="ExternalInput")
    g1h = nc.dram_tensor("g1h", [128, NCH_HI * CH // 16], i16, kind="ExternalInput")
    g2l = nc.dram_tensor("g2l", [128, NCH_LO * CH // 16], i16, kind="ExternalInput")
    g2h = nc.dram_tensor("g2h", [128, NCH_HI * CH // 16], i16, kind="ExternalInput")
    # negated local dst (within-block) per tile slot, rounds 1&2: [128, NT]
    nd1 = nc.dram_tensor("nd1", [128, NT], f32, kind="ExternalInput")
    nd2 = nc.dram_tensor("nd2", [128, NT], f32, kind="ExternalInput")
    w_in = nc.dram_tensor("w_in", [D, D], f32, kind="ExternalInput")
    b_in = nc.dram_tensor("b_in", [D, 1], f32, kind="ExternalInput")
    out_t = nc.dram_tensor("out_t", [D, NSLAB], f32, kind="ExternalOutput")

    with tile.TileContext(nc) as tc:
        with (
            tc.tile_pool(name="const", bufs=1) as cpool,
            tc.tile_pool(name="idx", bufs=1) as ipool,
            tc.tile_pool(name="ebuf", bufs=EBUFS) as epool,
            tc.tile_pool(name="sel", bufs=4) as spool,
            tc.tile_pool(name="fl", bufs=4) as fpool,
            tc.tile_pool(name="stg", bufs=1) as stpool,
            tc.tile_pool(name="ps", bufs=3, space="PSUM") as pspool,
            tc.tile_pool(name="ps2", bufs=2, space="PSUM") as ps2pool,
            tc.tile_pool(name="dram", bufs=1, space="DRAM") as dpool,
        ):
            gl_t = {r: ipool.tile([128, NCH_LO * CH // 16], i16,
                                  tag=f"g{r}l", name=f"g{r}l_t") for r in (1, 2)}
            gh_t = {r: ipool.tile([128, NCH_HI * CH // 16], i16,
                                  tag=f"g{r}h", name=f"g{r}h_t") for r in (1, 2)}
            nd_t = {r: ipool.tile([128, NT], f32, tag=f"nd{r}",
                                  name=f"nd{r}_t") for r in (1, 2)}
            nc.sync.dma_start(out=gl_t[1][:], in_=g1l[:])
            nc.sync.dma_start(out=gh_t[1][:], in_=g1h[:])
            nc.sync.dma_start(out=gl_t[2][:], in_=g2l[:])
            nc.sync.dma_start(out=gh_t[2][:], in_=g2h[:])
            nc.sync.dma_start(out=nd_t[1][:], in_=nd1[:])
            nc.sync.dma_start(out=nd_t[2][:], in_=nd2[:])

            w_t = cpool.tile([D, D], f32)
            b_t = cpool.tile([D, 1], f32)
            nc.sync.dma_start(out=w_t[:], in_=w_in[:])
            nc.sync.dma_start(out=b_t[:], in_=b_in[:])
            # neg_iota[p, r] = -r  (f32 ints <=128: exact)
            neg_iota = cpool.tile([128, 128], f32)
            nc.gpsimd.iota(neg_iota[:], pattern=[[-1, 128]], base=0,
                           channel_multiplier=0,
                           allow_small_or_imprecise_dtypes=True)

            h1part = dpool.tile([NSLAB, D], gdt)

            def one_round(rnd, table, table_hi):
                ebufs = {}
                qctr = [0]

                def ensure_chunk(stream, c):
                    key = (stream, c)
                    if key in ebufs:
                        return ebufs[key]
                    eb = epool.tile([128, TPC * D], gdt, tag="ebuf")
                    g = gl_t[rnd] if stream == 0 else gh_t[rnd]
                    tab = table if stream == 0 else table_hi
                    if PROBE_MODE == "seq_dma":
                        nc.sync.dma_start(out=eb[:], in_=tab[0:TPC * 128, :])
                    elif PROBE_MODE == "no_gather":
                        nc.vector.memset(eb[:], 0.0)
                    else:
                        nc.gpsimd.dma_gather(
                            eb[:].rearrange("p (n d) -> p n d", d=D),
                            tab,
                            g[:, c * (CH // 16):(c + 1) * (CH // 16)],
                            num_idxs=CH, num_idxs_reg=CH,
                            elem_size=D, elem_step=D,
                            queue_num=qctr[0] % NQ,
                            single_packet=SINGLE_PACKET)
                        qctr[0] += 1
                    ebufs[key] = eb
                    return eb

                stage = stpool.tile([128, NBIN * 128], gdt if rnd == 1 else f32,
                                    tag=f"stage{rnd}")
                for b in range(NBIN):
                    # grouped one-hot builds: S[edge, t, r] = (dstL[edge,t]==r)
                    S_lo = spool.tile([128, T_LO * 128], gdt, tag="Slo")
                    nc.vector.tensor_tensor(
                        out=S_lo[:].rearrange("p (t r) -> p t r", r=128),
                        in0=nd_t[rnd][:, b * T_LO:(b + 1) * T_LO]
                            .to_broadcast([128, T_LO, 128]),
                        in1=_mid_bcast(neg_iota[:], T_LO),
                        op=mybir.AluOpType.is_equal)
                    S_hi = spool.tile([128, T_HI * 128], gdt, tag="Shi")
                    c0 = NT_LO + b * T_HI
                    nc.vector.tensor_tensor(
                        out=S_hi[:].rearrange("p (t r) -> p t r", r=128),
                        in0=nd_t[rnd][:, c0:c0 + T_HI]
                            .to_broadcast([128, T_HI, 128]),
                        in1=_mid_bcast(neg_iota[:], T_HI),
                        op=mybir.AluOpType.is_equal)

                    ps = pspool.tile([128, 128], f32, tag="agg")
                    tiles = [(0, b * T_LO + j, S_lo, j) for j in range(T_LO)] + \
                            [(1, b * T_HI + j, S_hi, j) for j in range(T_HI)]
                    if PROBE_MODE == "gather_only":
                        for j, (stream, t, S, jj) in enumerate(tiles):
                            ensure_chunk(stream, t // TPC)
                        nc.tensor.matmul(ps[:], lhsT=S_lo[:, 0:128],
                                         rhs=S_hi[:, 0:128], start=True, stop=True)
                    else:
                        for j, (stream, t, S, jj) in enumerate(tiles):
                            eb = ensure_chunk(stream, t // TPC)
                            st = t % TPC
                            ebs = eb[:, st * D:(st + 1) * D]        # [edge, feat]
                            Ssl = S[:, jj * 128:(jj + 1) * 128]     # [edge, r]
                            nc.tensor.matmul(
                                ps[:],
                                # rnd1: out[r, feat] (stage layout direct);
                                # rnd2: out[feat, r] (feeds W matmul)
                                lhsT=Ssl if rnd == 1 else ebs,
                                rhs=ebs if rnd == 1 else Ssl,
                                start=(j == 0), stop=(j == len(tiles) - 1))
                    if rnd == 1:
                        nc.vector.tensor_copy(
                            stage[:, b * 128:(b + 1) * 128], ps[:])
                    else:
                        h_sb = fpool.tile([128, 128], f32, tag="hsb")
                        nc.scalar.copy(h_sb[:], ps[:])
                        o_ps = ps2pool.tile([128, 128], f32, tag="ops")
                        nc.tensor.matmul(o_ps[:], lhsT=w_t[:], rhs=h_sb[:],
                                         start=True, stop=True)
                        nc.vector.tensor_scalar_add(
                            stage[:, b * 128:(b + 1) * 128], o_ps[:], b_t[:, 0:1])
                # single flush DMA per round
                if rnd == 1:
                    nc.sync.dma_start(
                        out=h1part[:].rearrange("(b p) d -> p b d", p=128),
                        in_=stage[:].rearrange("p (b d) -> p b d", d=128))
                else:
                    nc.sync.dma_start(out=out_t[:], in_=stage[:])

            def allgather(rep, do_ag=True):
                # Shared output allows peers to RDMA-write directly (fast
                # path), but a Shared tensor takes exactly one writer inst:
                # allocate one per repeat iteration.
                h1full = dpool.tile([NFULL, D], gdt, addr_space="Shared",
                                    tag=f"h1full{rep}", name=f"h1full{rep}")
                if do_ag:
                    nc.gpsimd.collective_compute(
                        "AllGather", mybir.AluOpType.bypass,
                        replica_groups=[list(range(NCORES))],
                        ins=[h1part.opt()], outs=[h1full.opt()])
                return h1full

            if timing_loop is None:
                for _rep in range(repeat):
                    if mode == "agonly":
                        allgather(_rep)
                        continue
                    one_round(1, feature[:], feature[HSPLIT:, :])
                    h1full = allgather(_rep, do_ag=(mode == "full"))
                    one_round(2, h1full[:], h1full[HSPLIT:, :])
            elif mode == "loop_ag":
                # timing build incl. AG: one collective inst in the HW loop
                # (one writer inst for the Shared tensor, executed N times)
                h1full = allgather(0, do_ag=False)
                with tc.For_i(0, timing_loop, 1):
                    one_round(1, feature[:], feature[HSPLIT:, :])
                    nc.gpsimd.collective_compute(
                        "AllGather", mybir.AluOpType.bypass,
                        replica_groups=[list(range(NCORES))],
                        ins=[h1part.opt()], outs=[h1full.opt()])
                    one_round(2, h1full[:], h1full[HSPLIT:, :])
            else:
                # timing build: AllGather once (fills h1full), then loop rounds
                one_round(1, feature[:], feature[HSPLIT:, :])
                h1full = allgather(0)
                with tc.For_i(0, timing_loop, 1):
                    one_round(1, feature[:], feature[HSPLIT:, :])
                    one_round(2, h1full[:], h1full[HSPLIT:, :])
    nc.compile()
    return nc


def _build_nc_v2(repeat: int = 1, mode: str = "full"):
    """SBUF-resident bf16 table + transpose-mode SBUF-source gathers.

    Table layout (both rounds): row r lives at SBUF partition r%128, free
    byte offset (r//128)*256 — so a gather index is just the row id, and the
    round-1 stage buffer is already in this layout (flat flush, flat reload).
    Per 128-edge tile:
      Zmm:  z[e, f] = matmul(lhsT=E'[f, e], rhs=R)   (R = I round 1, W round 2
                                                       -> transpose [+ linear])
      copy: z_sb <- z (ACT, PSUM->SBUF)
      agg:  rnd1 ps[r, f] += matmul(lhsT=S[e, r], rhs=z_sb[e, f])
            rnd2 ps[of, r] += matmul(lhsT=z_sb[e, of], rhs=S[e, r])
    """
    nc = bacc.Bacc(
        "TRN2",
        target_bir_lowering=False,
        debug=False,
        num_devices=NCORES,
        num_swdge_queues=NQ,
    )
    f32, i16, bf16 = mybir.dt.float32, mybir.dt.int16, mybir.dt.bfloat16
    NRANK = NFULL // 128             # 392 ranks of 128 rows

    feature = nc.dram_tensor("feature", [128, NFULL], bf16, kind="ExternalInput")
    g1l = nc.dram_tensor("g1l", [128, NCH_LO * CH // 16], i16, kind="ExternalInput")
    g1h = nc.dram_tensor("g1h", [128, NCH_HI * CH // 16], i16, kind="ExternalInput")
    g2l = nc.dram_tensor("g2l", [128, NCH_LO * CH // 16], i16, kind="ExternalInput")
    g2h = nc.dram_tensor("g2h", [128, NCH_HI * CH // 16], i16, kind="ExternalInput")
    nd1 = nc.dram_tensor("nd1", [128, NT], f32, kind="ExternalInput")
    nd2 = nc.dram_tensor("nd2", [128, NT], f32, kind="ExternalInput")
    w_in = nc.dram_tensor("w_in", [D, D], f32, kind="ExternalInput")
    b_in = nc.dram_tensor("b_in", [D, 1], f32, kind="ExternalInput")
    out_t = nc.dram_tensor("out_t", [D, NSLAB], f32, kind="ExternalOutput")

    with tile.TileContext(nc) as tc:
        with (
            tc.tile_pool(name="const", bufs=1) as cpool,
            tc.tile_pool(name="idx", bufs=1) as ipool,
            tc.tile_pool(name="tblp", bufs=1) as tpool,
            tc.tile_pool(name="ebuf", bufs=6) as epool,
            tc.tile_pool(name="zb", bufs=6) as zpool,
            tc.tile_pool(name="sel", bufs=4) as spool,
            tc.tile_pool(name="stg", bufs=1) as stpool,
            tc.tile_pool(name="psz", bufs=4, space="PSUM") as pszpool,
            tc.tile_pool(name="ps", bufs=3, space="PSUM") as pspool,
            tc.tile_pool(name="dram", bufs=1, space="DRAM") as dpool,
        ):
            gl_t = {r: ipool.tile([128, NCH_LO * CH // 16], i16,
                                  tag=f"g{r}l", name=f"g{r}l_t") for r in (1, 2)}
            gh_t = {r: ipool.tile([128, NCH_HI * CH // 16], i16,
                                  tag=f"g{r}h", name=f"g{r}h_t") for r in (1, 2)}
            nd_t = {r: ipool.tile([128, NT], f32, tag=f"nd{r}",
                                  name=f"nd{r}_t") for r in (1, 2)}
            nc.sync.dma_start(out=gl_t[1][:], in_=g1l[:])
            nc.sync.dma_start(out=gh_t[1][:], in_=g1h[:])
            nc.sync.dma_start(out=gl_t[2][:], in_=g2l[:])
            nc.sync.dma_start(out=gh_t[2][:], in_=g2h[:])
            nc.sync.dma_start(out=nd_t[1][:], in_=nd1[:])
            nc.sync.dma_start(out=nd_t[2][:], in_=nd2[:])

            w_t = cpool.tile([D, D], f32)
            nc.sync.dma_start(out=w_t[:], in_=w_in[:])
            w_bf = cpool.tile([D, D], bf16)
            nc.vector.tensor_copy(w_bf[:], w_t[:])
            b_t = cpool.tile([D, 1], f32)
            nc.sync.dma_start(out=b_t[:], in_=b_in[:])
            ident = cpool.tile([128, 128], bf16)
            make_identity(nc, ident[:])
            neg_iota = cpool.tile([128, 128], f32)
            nc.gpsimd.iota(neg_iota[:], pattern=[[-1, 128]], base=0,
                           channel_multiplier=0,
                           allow_small_or_imprecise_dtypes=True)

            h1part = dpool.tile([128, NSLAB], bf16)
            h1full = dpool.tile([NCORES * 128, NSLAB], bf16)
            tbl = tpool.tile([128, NFULL], bf16)

            def load_table(rnd):
                if rnd == 1:
                    nc.sync.dma_start(out=tbl[:], in_=feature[:])
                else:
                    nc.sync.dma_start(
                        out=tbl[:].rearrange("p (c f) -> p c f", c=NCORES),
                        in_=h1full[:].rearrange("(c p) f -> p c f", p=128))

            def one_round(rnd):
                ebufs = {}
                qctr = [0]

                def ensure_chunk(stream, c):
                    key = (stream, c)
                    if key in ebufs:
                        return ebufs[key]
                    eb = epool.tile([128, CH], bf16, tag="ebuf")
                    if PROBE_MODE == "no_gather":
                        nc.vector.memset(eb[:], 0.0)
                    else:
                        g = gl_t[rnd] if stream == 0 else gh_t[rnd]
                        src = tbl[:] if stream == 0 else tbl[:, HSPLIT:]
                        nc.gpsimd.dma_gather(
                            eb[:].rearrange("p (o e) -> p o e", o=1),
                            src,
                            g[:, c * (CH // 16):(c + 1) * (CH // 16)],
                            num_idxs=CH, num_idxs_reg=CH,
                            elem_size=D, transpose=True,
                            sbuf_tokens_per_rank=128,
                            sbuf_free_dim_per_rank=256,
                            queue_num=qctr[0] % NQ,
                            single_packet=SINGLE_PACKET)
                        qctr[0] += 1
                    ebufs[key] = eb
                    return eb

                stage = stpool.tile([128, NSLAB], bf16 if rnd == 1 else f32,
                                    tag=f"stage{rnd}")
                for b in range(NBIN):
                    S_lo = spool.tile([128, T_LO * 128], bf16, tag="Slo")
                    nc.vector.tensor_tensor(
                        out=S_lo[:].rearrange("p (t r) -> p t r", r=128),
                        in0=nd_t[rnd][:, b * T_LO:(b + 1) * T_LO]
                            .to_broadcast([128, T_LO, 128]),
                        in1=_mid_bcast(neg_iota[:], T_LO),
                        op=mybir.AluOpType.is_equal)
                    S_hi = spool.tile([128, T_HI * 128], bf16, tag="Shi")
                    c0 = NT_LO + b * T_HI
                    nc.vector.tensor_tensor(
                        out=S_hi[:].rearrange("p (t r) -> p t r", r=128),
                        in0=nd_t[rnd][:, c0:c0 + T_HI]
                            .to_broadcast([128, T_HI, 128]),
                        in1=_mid_bcast(neg_iota[:], T_HI),
                        op=mybir.AluOpType.is_equal)

                    ps = pspool.tile([128, 128], f32, tag="agg")
                    tiles = [(0, b * T_LO + j, S_lo, j) for j in range(T_LO)] + \
                            [(1, b * T_HI + j, S_hi, j) for j in range(T_HI)]
                    if PROBE_MODE == "gather_only":
                        for j, (stream, t, S, jj) in enumerate(tiles):
                            ensure_chunk(stream, t // TPC)
                        nc.tensor.matmul(ps[:], lhsT=S_lo[:, 0:128],
                                         rhs=S_hi[:, 0:128], start=True,
                                         stop=True)
                        tiles = []
                    for j, (stream, t, S, jj) in enumerate(tiles):
                        eb = ensure_chunk(stream, t // TPC)
                        st = t % TPC
                        zp = pszpool.tile([128, 128], f32, tag="zps")
                        nc.tensor.matmul(
                            zp[:],
                            lhsT=eb[:, st * 128:(st + 1) * 128],
                            rhs=ident[:] if rnd == 1 else w_bf[:],
                            start=True, stop=True)
                        z_sb = zpool.tile([128, 128], bf16, tag="zsb")
                        nc.scalar.copy(z_sb[:], zp[:])
                        Ssl = S[:, jj * 128:(jj + 1) * 128]
                        nc.tensor.matmul(
                            ps[:],
                            lhsT=Ssl if rnd == 1 else z_sb[:],
                            rhs=z_sb[:] if rnd == 1 else Ssl,
                            start=(j == 0), stop=(j == len(tiles) - 1))
                    if rnd == 1:
                        nc.vector.tensor_copy(
                            stage[:, b * 128:(b + 1) * 128], ps[:])
                    else:
                        nc.vector.tensor_scalar_add(
                            stage[:, b * 128:(b + 1) * 128], ps[:], b_t[:, 0:1])
                if rnd == 1:
                    nc.sync.dma_start(out=h1part[:], in_=stage[:])
                else:
                    nc.sync.dma_start(out=out_t[:], in_=stage[:])

            def allgather():
                nc.gpsimd.collective_compute(
                    "AllGather", mybir.AluOpType.bypass,
                    replica_groups=[list(range(NCORES))],
                    ins=[h1part.opt()], outs=[h1full.opt()])

            for _rep in range(repeat):
                if mode == "agonly":
                    allgather()
                    continue
                load_table(1)
                one_round(1)
                if mode == "full":
                    allgather()
                    load_table(2)
                one_round(2)
    nc.compile()
    return nc


VARIANT = "v1"
_NC_CACHE: dict = {}


def get_nc(repeat: int = 1):
    key = (VARIANT, repeat, NQ, GATHER_BF16, SINGLE_PACKET)
    if key not in _NC_CACHE:
        _NC_CACHE[key] = (_build_nc_v2(repeat) if VARIANT == "v2"
                          else _build_nc(repeat))
    return _NC_CACHE[key]


def _wrap_idx(idx: np.ndarray) -> np.ndarray:
    """[n] -> [128, n//16] int16 wrapped layout (16-partition wrap, 8x
    replicated for the Q7 cores)."""
    n = idx.shape[0]
    w = idx.reshape(n // 16, 16).T.astype(np.int16)
    return np.ascontiguousarray(np.tile(w, (8, 1)))


def _pack_bins(deg_lo: np.ndarray, deg: np.ndarray):
    """Balanced snake packing of PER nodes into NBIN bins (<=128 nodes each).
    Returns perm: perm[orig_local] = bin*128 + slot."""
    order = np.argsort(-(deg_lo * 2 + deg), kind="stable")
    bins = [[] for _ in range(NBIN)]
    pos, fwd = 0, True
    for n in order:
        for _ in range(NBIN + 1):
            if len(bins[pos]) < 128:
                break
            pos, fwd = _step(pos, fwd, NBIN)
        bins[pos].append(n)
        pos, fwd = _step(pos, fwd, NBIN)
    perm = np.empty(PER, np.int64)
    for bi, members in enumerate(bins):
        for sl, n in enumerate(members):
            perm[n] = bi * 128 + sl
    return perm


def _step(pos, fwd, nbins):
    if fwd:
        if pos + 1 >= nbins:
            return pos, False
        return pos + 1, True
    if pos - 1 < 0:
        return pos, True
    return pos - 1, False


def _build_round_tensors(g_src: np.ndarray, dslot: np.ndarray):
    """Per-edge gather ids + permuted local dst slots -> (gl, gh, nd)."""
    blk = dslot >> 7
    dstL = dslot & 127
    is_lo = g_src < HSPLIT

    g_lo = np.zeros(NCH_LO * CH, np.int64)
    g_hi = np.zeros(NCH_HI * CH, np.int64)
    ndl = np.full((128, NT), 1.0, np.float32)   # -dstL; pad = +1 (never matches)
    for b in range(NBIN):
        in_b = blk == b
        for stream, (g_arr, t0, t_n, base_off) in enumerate(
                [(g_lo, b * T_LO, T_LO, 0), (g_hi, b * T_HI, T_HI, NT_LO)]):
            m = in_b & (is_lo if stream == 0 else ~is_lo)
            gs = g_src[m] - (0 if stream == 0 else HSPLIT)
            dl = dstL[m]
            cnt = gs.shape[0]
            assert cnt <= t_n * 128, (b, stream, cnt, t_n * 128)
            sl = t0 * 128
            g_arr[sl: sl + cnt] = gs
            col = np.arange(cnt) // 128 + t0 + base_off
            row = np.arange(cnt) % 128
            ndl[row, col] = -dl.astype(np.float32)
    return (_wrap_idx(g_lo), _wrap_idx(g_hi), ndl)


def prep_core_inputs(feature, W, b, src, dst):
    feature = np.ascontiguousarray(np.asarray(feature, dtype=np.float32))
    W = np.ascontiguousarray(np.asarray(W, dtype=np.float32))
    b = np.asarray(b, dtype=np.float32).reshape(D, 1)
    src = np.asarray(src).astype(np.int64)
    dst = np.asarray(dst).astype(np.int64)

    owner = dst // PER
    deg_lo_all = np.bincount(dst[src < HSPLIT], minlength=N_NODES)
    deg_all = np.bincount(dst, minlength=N_NODES)
    perms = []
    for c in range(NCORES):
        dlo = deg_lo_all[c * PER:(c + 1) * PER]
        dg = deg_all[c * PER:(c + 1) * PER]
        perms.append(_pack_bins(dlo.astype(np.int64), dg.astype(np.int64)))

    permg = np.empty(N_NODES, np.int64)
    for c in range(NCORES):
        permg[c * PER:(c + 1) * PER] = c * NSLAB + perms[c]

    if VARIANT == "v2":
        # wrapped layout: feat_w[p, (r//128)*128 + f] = feature[r, f], r%128==p
        fpad = np.zeros((NFULL, D), np.float32)
        fpad[:N_NODES] = feature
        feat_in = np.ascontiguousarray(
            fpad.reshape(NFULL // 128, 128, D).transpose(1, 0, 2)
                .reshape(128, NFULL)).astype(mybir.dt.np(mybir.dt.bfloat16))
    else:
        feat_in = (feature.astype(mybir.dt.np(mybir.dt.bfloat16))
                   if GATHER_BF16 else feature)
    in_maps = []
    for c in range(NCORES):
        sel = owner == c
        es, ed = src[sel], dst[sel] - c * PER
        dslot = perms[c][ed]
        g1l_, g1h_, nd1_ = _build_round_tensors(es, dslot)
        g2l_, g2h_, nd2_ = _build_round_tensors(permg[es], dslot)
        in_maps.append({
            "feature": feat_in,
            "g1l": g1l_, "g1h": g1h_, "nd1": nd1_,
            "g2l": g2l_, "g2h": g2h_, "nd2": nd2_,
            "w_in": W, "b_in": b,
        })
    return in_maps, perms


def assemble(results, perms) -> np.ndarray:
    out = np.empty((N_NODES, D), np.float32)
    for c in range(NCORES):
        ot = np.asarray(results[c]["out_t"])       # [D, NSLAB]
        out[c * PER:(c + 1) * PER, :] = ot.T[perms[c], :]
    return out


def kernel(feature, W, b, src, dst) -> np.ndarray:
    nc = get_nc(repeat=1)
    in_maps, perms = prep_core_inputs(feature, W, b, src, dst)
    res = run_bass_kernel_spmd(nc, in_maps, core_ids=list(range(NCORES)))
    return assemble(res.results, perms)

